# revision 1
# baseline (speedup 1.0000x reference)
"""Trainium2 Bass kernel v3: entmax-1.5 along last dim of x[8,16,1024,1024] f32.

Row-parallel over 8 NeuronCores. Validated in numpy against the float64
sort reference on 32k real rows: max |y-yref| = 2.55e-3 (gate 2e-2).

Algorithm per row (d=1024), h = f16(x/2), tau in h-space:
  seed: closed-form k=8 threshold over the 8 chunk maxima of h (ACT Sqrt).
  2 support iterations with threshold quantized to f16 and used
  consistently (off-support terms cancel exactly):
    k  = sum(h > t16)                  DVE is_gt + accum
    r  = max(h, t16), A1 = sum r       DVE max + accum
    s2 = sum (r - t16)^2               ACT Square(r, bias=-t16) + accum
    s1 = A1 - 1024 t16; theta = 2-step-Newton root of
         k th^2 - 2 s1 th + (s2-1), clamped to [-2, s1/k]  (all-DVE chain)
  final: r3 = max(h, t16); y = Square(r3 - t16) -> f32.

HW lessons encoded here (all measured on these cores):
  - GPSIMD streaming is ~20x slower than modeled -> Pool unused.
  - 4-dim DMA access patterns (512B descriptors) degrade badly with 8 cores
    running concurrently -> v1-style "(a p) m -> p a m" 4KB descriptors.
  - accum_out ops run ~1us (not 327ns); ACT Square with bias-AP+accum
    ~2.6us; plain streaming ops are fast.
  - Cross-engine dependency hops in the t-update chains are expensive ->
    chains are all-DVE (2-step Newton instead of sqrt; ACT Sqrt only in
    the per-group seed chain), and the two groups of an emission pair
    share one [P, 2G] chain.
"""

import sys

sys.path.insert(0, "/opt/trn_rl_repo")
sys.path.insert(0, "/opt/trn_rl_repo/concourse")

from contextlib import ExitStack

import numpy as np

D = 1024
P = 128
N_CORES = 8


def build_program(n_rows, group_tiles=16, dma_batch=4, debug=False, reps=1,
                  xp_bufs=4, hp_mult=2, tr_bufs=2, yp_bufs=2, smp_bufs=2,
                  ablate=()):
    import concourse.bacc as bacc
    import concourse.tile as tile
    from concourse import mybir

    F32 = mybir.dt.float32
    F16 = mybir.dt.float16
    ALU = mybir.AluOpType
    ACTF = mybir.ActivationFunctionType
    AX = mybir.AxisListType

    G = group_tiles
    B = dma_batch
    nb = G // B
    T = n_rows // P
    assert n_rows % P == 0 and T % G == 0 and G % B == 0
    n_groups = T // G

    nc = bacc.Bacc(
        "TRN2", target_bir_lowering=False, debug=debug, enable_asserts=False
    )
    x = nc.dram_tensor("x", [n_rows, D], F32, kind="ExternalInput").ap()
    y = nc.dram_tensor("y", [n_rows, D], F32, kind="ExternalOutput").ap()

    with tile.TileContext(nc) as tc, ExitStack() as ctx:
        xp = ctx.enter_context(tc.tile_pool(name="xp", bufs=xp_bufs))
        hp = ctx.enter_context(tc.tile_pool(name="hp", bufs=hp_mult * nb))
        c8p = ctx.enter_context(tc.tile_pool(name="c8p", bufs=2))
        rt = ctx.enter_context(tc.tile_pool(name="rt", bufs=tr_bufs))
        r32 = ctx.enter_context(tc.tile_pool(name="r32", bufs=2))
        kt = ctx.enter_context(tc.tile_pool(name="kt", bufs=tr_bufs))
        sqt = ctx.enter_context(tc.tile_pool(name="sqt", bufs=tr_bufs))
        yp = ctx.enter_context(tc.tile_pool(name="yp", bufs=yp_bufs))
        smp = ctx.enter_context(tc.tile_pool(name="smp", bufs=smp_bufs))

        state = {}
        src = {"t": x}

        # pair-level state: W = columns of the merged chain (2G normally)
        def hview(pair_state, q):
            """flat [P, 1024] f16 view of pair-tile q (0..W-1)."""
            hb = pair_state["hb"][q // B]
            return hb[:, q % B, :]

        def stage_load_cast_seed(pair):
            """Load, cast, chunk-max and seed for ALL groups of the pair;
            one merged [P, W] chain (W = 16*len(pair))."""
            W = G * len(pair)
            ps = {"hb": [], "W": W, "pair": pair}
            c8 = c8p.tile([P, W, 8], F32, tag="c8")
            for gi, g in enumerate(pair):
                g_row0 = g * G * P
                for b in range(nb):
                    xt = xp.tile([P, B, D], F32, tag="x")
                    r0 = g_row0 + b * B * P
                    if "no_dma_in" not in ablate:
                        nc.sync.dma_start(
                            xt[:],
                            src["t"][r0 : r0 + B * P, :].rearrange(
                                "(a p) m -> p a m", p=P
                            ),
                        )
                    else:
                        nc.vector.memset(xt[:, 0:1, 0:8], 0.5)
                    ht = hp.tile([P, B, D], F16, tag="h")
                    nc.vector.tensor_scalar(
                        ht[:], xt[:], 0.5, None, op0=ALU.mult
                    )
                    ps["hb"].append(ht)
                    for jj in range(B):
                        q = gi * G + b * B + jj
                        nc.vector.tensor_reduce(
                            c8[:, q, :],
                            ht[:, jj, :].rearrange("p (c e) -> p c e", e=128),
                            axis=AX.X,
                            op=ALU.max,
                        )

            # merged seed chain (k=8 closed form, ACT Sqrt for the root)
            m = smp.tile([P, W], F32, tag="m")
            nc.vector.tensor_reduce(m[:], c8[:], axis=AX.X, op=ALU.max)
            S = smp.tile([P, W], F32, tag="S")
            nc.vector.tensor_reduce(S[:], c8[:], axis=AX.X, op=ALU.add)
            c8sq = c8p.tile([P, W, 8], F32, tag="c8sq")
            nc.vector.tensor_tensor(c8sq[:], c8[:], c8[:], op=ALU.mult)
            Q = smp.tile([P, W], F32, tag="Q")
            nc.vector.tensor_reduce(Q[:], c8sq[:], axis=AX.X, op=ALU.add)
            mm = smp.tile([P, W], F32, tag="mm")
            nc.vector.tensor_scalar_mul(mm[:], m[:], -8.0)
            s1 = smp.tile([P, W], F32, tag="s1")
            nc.vector.tensor_tensor(s1[:], S[:], mm[:], op=ALU.add)
            mS = smp.tile([P, W], F32, tag="mS")
            nc.vector.tensor_tensor(mS[:], m[:], S[:], op=ALU.mult)
            m2 = smp.tile([P, W], F32, tag="m2")
            nc.vector.tensor_tensor(m2[:], m[:], m[:], op=ALU.mult)
            a1 = smp.tile([P, W], F32, tag="a1")
            nc.vector.tensor_scalar_mul(a1[:], mS[:], -2.0)
            a2 = smp.tile([P, W], F32, tag="a2")
            nc.vector.tensor_scalar_mul(a2[:], m2[:], 8.0)
            s2a = smp.tile([P, W], F32, tag="s2a")
            nc.vector.tensor_tensor(s2a[:], Q[:], a1[:], op=ALU.add)
            s2 = smp.tile([P, W], F32, tag="s2")
            nc.vector.tensor_tensor(s2[:], s2a[:], a2[:], op=ALU.add)
            q_ = smp.tile([P, W], F32, tag="q")
            nc.vector.tensor_tensor(q_[:], s1[:], s1[:], op=ALU.mult)
            b1 = smp.tile([P, W], F32, tag="b1")
            nc.vector.tensor_scalar(
                b1[:], s2[:], -8.0, 8.0, op0=ALU.mult, op1=ALU.add
            )
            d0 = smp.tile([P, W], F32, tag="d0")
            nc.vector.tensor_tensor(d0[:], q_[:], b1[:], op=ALU.add)
            dn = smp.tile([P, W], F32, tag="dn")
            nc.vector.tensor_scalar_max(dn[:], d0[:], 1e-30)
            root = smp.tile([P, W], F32, tag="root")
            nc.scalar.activation(root[:], dn[:], ACTF.Sqrt)
            num = smp.tile([P, W], F32, tag="num")
            nc.vector.tensor_tensor(num[:], s1[:], root[:], op=ALU.subtract)
            th = smp.tile([P, W], F32, tag="th")
            nc.vector.tensor_scalar_mul(th[:], num[:], 0.125)
            t0 = smp.tile([P, W], F32, tag="t0")
            nc.vector.tensor_tensor(t0[:], m[:], th[:], op=ALU.add)
            ps["t"] = t0
            return ps

        def q16(ps):
            W = ps["W"]
            if "fixed_t" in ablate:
                t16f = smp.tile([P, W], F32, tag="t16f")
                nc.vector.memset(t16f[:], 1.25)
                tb = smp.tile([P, W], F32, tag="tb")
                nc.vector.memset(tb[:], -1.25)
                return t16f, tb
            t16h = smp.tile([P, W], F16, tag="t16h")
            nc.vector.tensor_scalar_mul(t16h[:], ps["t"][:], 1.0)
            t16f = smp.tile([P, W], F32, tag="t16f")
            nc.vector.tensor_scalar_mul(t16f[:], t16h[:], 1.0)
            tb = smp.tile([P, W], F32, tag="tb")
            nc.vector.tensor_scalar_mul(tb[:], t16f[:], -1.0)
            return t16f, tb

        def stage_iter(ps, it):
            W = ps["W"]
            t16f, tb = q16(ps)
            if it == 0:
                K = smp.tile([P, W], F32, tag="K")
                ps["K"] = K
            else:
                K = ps["K"]  # reuse iter-1 support count (validated 4.5e-3)
            A1 = smp.tile([P, W], F32, tag="A1")
            S2 = smp.tile([P, W], F32, tag="S2")
            for q in range(W):
                h_q = hview(ps, q)
                t_col = t16f[:, q : q + 1]
                if it == 0:
                    kct = kt.tile([P, D], F16, tag="k")
                    nc.vector.tensor_scalar(
                        kct[:], h_q, t_col, None,
                        op0=ALU.is_gt, op1=ALU.add,
                        accum_out=K[:, q : q + 1],
                    )
                rte = rt.tile([P, D], F16, tag="r")
                nc.vector.tensor_scalar(
                    rte[:], h_q, t_col, None,
                    op0=ALU.max, op1=ALU.add,
                    accum_out=A1[:, q : q + 1],
                )
                sqe = sqt.tile([P, D], F16, tag="sq")
                nc.scalar.activation(
                    sqe[:], rte[:], ACTF.Square,
                    scale=1.0, bias=tb[:, q : q + 1],
                    accum_out=S2[:, q : q + 1],
                )
            if "fixed_t" in ablate:
                return
            # all-DVE 2-step-Newton update, clamped to [-2, s1/k]
            tm = smp.tile([P, W], F32, tag="tm")
            nc.vector.tensor_scalar_mul(tm[:], t16f[:], -1024.0)
            s1 = smp.tile([P, W], F32, tag="s1i")
            nc.vector.tensor_tensor(s1[:], A1[:], tm[:], op=ALU.add)
            s1g = smp.tile([P, W], F32, tag="s1g")
            nc.vector.tensor_scalar_max(s1g[:], s1[:], 1e-6)
            g1 = smp.tile([P, W], F32, tag="g1")
            nc.vector.tensor_scalar(g1[:], S2[:], -1.0, None, op0=ALU.add)
            rp = smp.tile([P, W], F32, tag="rp")
            nc.vector.reciprocal(rp[:], s1g[:])
            a_ = smp.tile([P, W], F32, tag="a_")
            nc.vector.tensor_tensor(a_[:], g1[:], rp[:], op=ALU.mult)
            th1 = smp.tile([P, W], F32, tag="th1")
            nc.vector.tensor_scalar_mul(th1[:], a_[:], 0.5)
            e = smp.tile([P, W], F32, tag="e")
            nc.vector.tensor_tensor(e[:], K[:], th1[:], op=ALU.mult)
            c_ = smp.tile([P, W], F32, tag="c_")
            nc.vector.tensor_tensor(c_[:], e[:], s1g[:], op=ALU.subtract)
            # qv = th1*(e - 2 s1) + g = th1*(c_ - s1) + g
            c2 = smp.tile([P, W], F32, tag="c2")
            nc.vector.tensor_tensor(c2[:], c_[:], s1g[:], op=ALU.subtract)
            u_ = smp.tile([P, W], F32, tag="u_")
            nc.vector.tensor_tensor(u_[:], th1[:], c2[:], op=ALU.mult)
            qv = smp.tile([P, W], F32, tag="qv")
            nc.vector.tensor_tensor(qv[:], u_[:], g1[:], op=ALU.add)
            # qp = 2*(e - s1) = 2*c_
            qp = smp.tile([P, W], F32, tag="qp")
            nc.vector.tensor_scalar_mul(qp[:], c_[:], 2.0)
            rq = smp.tile([P, W], F32, tag="rq")
            nc.vector.reciprocal(rq[:], qp[:])
            d_ = smp.tile([P, W], F32, tag="d_")
            nc.vector.tensor_tensor(d_[:], qv[:], rq[:], op=ALU.mult)
            th2 = smp.tile([P, W], F32, tag="th2")
            nc.vector.tensor_tensor(th2[:], th1[:], d_[:], op=ALU.subtract)
            kg = smp.tile([P, W], F32, tag="kg")
            nc.vector.tensor_scalar_max(kg[:], K[:], 1.0)
            kinv = smp.tile([P, W], F32, tag="kinv")
            nc.vector.reciprocal(kinv[:], kg[:])
            thv = smp.tile([P, W], F32, tag="thv")
            nc.vector.tensor_tensor(thv[:], s1g[:], kinv[:], op=ALU.mult)
            thc = smp.tile([P, W], F32, tag="thc")
            nc.vector.tensor_tensor(thc[:], th2[:], thv[:], op=ALU.min)
            thc2 = smp.tile([P, W], F32, tag="thc2")
            nc.vector.tensor_scalar_max(thc2[:], thc[:], -2.0)
            t_new = smp.tile([P, W], F32, tag="tn")
            nc.vector.tensor_tensor(t_new[:], ps["t"][:], thc2[:], op=ALU.add)
            ps["t"] = t_new

        def stage_final(ps):
            t16f, tb = q16(ps)
            W = ps["W"]
            # final reads a fresh f32 copy of the input (h tiles are already
            # released after iter 2 -> deeper cross-pair overlap), and the
            # f32 final improves y precision over the f16-h path.
            t32 = smp.tile([P, W], F32, tag="t32")
            nc.vector.tensor_scalar_mul(t32[:], t16f[:], 2.0)
            for gi, g in enumerate(ps["pair"]):
                g_row0 = g * G * P
                for b in range(nb):
                    r0 = g_row0 + b * B * P
                    xt2 = xp.tile([P, B, D], F32, tag="x")
                    if "no_dma_in" not in ablate:
                        nc.sync.dma_start(
                            xt2[:],
                            src["t"][r0 : r0 + B * P, :].rearrange(
                                "(a p) m -> p a m", p=P
                            ),
                        )
                    else:
                        nc.vector.memset(xt2[:, 0:1, 0:8], 0.5)
                    yt = yp.tile([P, B, D], F32, tag="y")
                    for jj in range(B):
                        q = gi * G + b * B + jj
                        rte = r32.tile([P, D], F32, tag="r3")
                        nc.vector.tensor_scalar(
                            rte[:], xt2[:, jj, :], t32[:, q : q + 1], None,
                            op0=ALU.max,
                        )
                        nc.scalar.activation(
                            yt[:, jj, :], rte[:], ACTF.Square,
                            scale=0.5, bias=tb[:, q : q + 1],
                        )
                    if "no_dma_out" not in ablate:
                        nc.sync.dma_start(
                            y[r0 : r0 + B * P, :].rearrange(
                                "(a p) m -> p a m", p=P
                            ),
                            yt[:],
                        )

        for rep in range(reps):
            if rep == 1:
                src["t"] = y
            for p0 in range(0, n_groups, 2):
                pair = [g for g in (p0, p0 + 1) if g < n_groups]
                ps = stage_load_cast_seed(pair)
                for it in range(2):
                    stage_iter(ps, it)
                stage_final(ps)

    nc.compile()
    return nc


_PROGRAM = None
_PROGRAM_ROWS = None


def _get_program(rows_per_core):
    global _PROGRAM, _PROGRAM_ROWS
    if _PROGRAM is None or _PROGRAM_ROWS != rows_per_core:
        _PROGRAM = build_program(rows_per_core)
        _PROGRAM_ROWS = rows_per_core
    return _PROGRAM


def run_sharded(flat_x, trace=False):
    from concourse.bass_utils import run_bass_kernel_spmd

    n_rows = flat_x.shape[0]
    rows_per = n_rows // N_CORES
    assert rows_per * N_CORES == n_rows
    nc = _get_program(rows_per)
    in_maps = [
        {"x": np.ascontiguousarray(flat_x[i * rows_per : (i + 1) * rows_per])}
        for i in range(N_CORES)
    ]
    res = run_bass_kernel_spmd(nc, in_maps, list(range(N_CORES)), trace=trace)
    y = np.concatenate([res.results[i]["y"] for i in range(N_CORES)], axis=0)
    return y, res


def kernel(x):
    x = np.ascontiguousarray(np.asarray(x), dtype=np.float32)
    orig_shape = x.shape
    flat = x.reshape(-1, D)
    y, _ = run_sharded(flat)
    return y.reshape(orig_shape)

